# revision 23
# baseline (speedup 1.0000x reference)
"""Transformer encoder layer (LN -> MHA -> residual -> LN -> MLP -> residual)
on 8 Trainium2 NeuronCores.

Sharding: token-parallel over the 4096 (batch*seq) tokens, 512 query-tokens
per core.  Each core computes K/V projections only for its OWN 512 tokens;
the full 2048-token K/V per batch is assembled with two AllGather
collectives (bf16, ~1 MB each) across the 4-core group sharing a batch.
Collectives run on TOPSP/SDMA and overlap with the Q projection.

On-chip layout: activations are kept feature-major ("transposed", [d, token])
so every matmul contracts along the partition dim.  Weights are pre-arranged
on the host so every weight DMA is one contiguous run per partition (DMA
descriptor count is the latency driver, not bytes).  Matmul operands are
bf16; accumulation stays fp32 in PSUM.  Softmax is computed unnormalized
(scores are bounded so plain exp is safe and algebraically identical); the
denominator comes from a ones-column interleaved into V before the gather,
and each head's attention accumulator stays resident in one PSUM bank
across all 16 k-tiles.  The score matmuls run one (wave, chunk) step ahead
of the attn@V matmuls so the exp's on the scalar engine pipeline behind
full-speed PE bursts.

LayerNorm gains/biases are folded into the following projections on the host
(exact algebra: (g*xhat+b) @ W = xhat @ (diag(g) W) + b @ W).
"""

import numpy as np
import ml_dtypes

import concourse.bass as bass
import concourse.mybir as mybir
from concourse import bacc
from concourse.tile import TileContext
from concourse.bass_utils import run_bass_kernel_spmd
from concourse.masks import make_identity

F32 = mybir.dt.float32
F32R = mybir.dt.float32r
BF16 = mybir.dt.bfloat16
FP8 = mybir.dt.float8e4
PM = mybir.MatmulPerfMode
AF = mybir.ActivationFunctionType
ALU = mybir.AluOpType

B, S, D = 2, 2048, 1024
H, HD = 16, 64
DFF = 4 * D
NCORES = 8
QT = 512            # query tokens per core
EPS = 1e-5
RG = [[0, 1, 2, 3], [4, 5, 6, 7]]  # replica groups (one per batch)


def _ln_stats(nc, lnp, eps, xt_a, xt_b):
    """bn stats over two [128, 512] token half-tiles -> (-mu*rstd, rstd)."""
    stats = lnp.tile([128, 2, 6], F32, tag="ln_st")
    nc.vector.bn_stats(stats[:, 0, :], xt_a)
    nc.vector.bn_stats(stats[:, 1, :], xt_b)
    mv = lnp.tile([128, 2], F32, tag="ln_mv")
    nc.vector.bn_aggr(mv, stats)
    sd = lnp.tile([128, 1], F32, tag="ln_sd")
    nc.scalar.activation(sd, mv[:, 1:2], AF.Sqrt, bias=eps[:, 0:1])
    rstd = lnp.tile([128, 1], F32, tag="ln_rs")
    nc.vector.reciprocal(rstd, sd)
    mr = lnp.tile([128, 2], F32R, tag="ln_mr")
    nc.vector.tensor_scalar(
        mr[:, 0:1], mv[:, 0:1], rstd, -1.0, ALU.mult, ALU.mult
    )
    nc.vector.tensor_copy(mr[:, 1:2], rstd)
    return mr


def _build():
    nc = bacc.Bacc(None, target_bir_lowering=False, num_devices=NCORES)

    XQ = nc.declare_dram_parameter("xq", [QT, D], F32, isOutput=False)
    XQT = nc.declare_dram_parameter("xqt", [D, QT], F32, isOutput=False)
    # host-prearranged weights: one contiguous run per partition per load
    WQR = nc.declare_dram_parameter("wqr", [8, 128, 8, 128], BF16, isOutput=False)
    WKR = nc.declare_dram_parameter("wkr", [8, 128, 8, 128], BF16, isOutput=False)
    WVR = nc.declare_dram_parameter("wvr", [2, 128, 8, 512], BF16, isOutput=False)
    WO = nc.declare_dram_parameter("wo", [D, D], BF16, isOutput=False)
    W1R = nc.declare_dram_parameter("w1r", [32, 128, 8, 128], FP8, isOutput=False)
    W2R = nc.declare_dram_parameter("w2r", [16, 128, 2, D], FP8, isOutput=False)
    BQ = nc.declare_dram_parameter("bq", [D], F32, isOutput=False)
    BK = nc.declare_dram_parameter("bk", [D], F32, isOutput=False)
    BV = nc.declare_dram_parameter("bv", [D], F32, isOutput=False)
    BO = nc.declare_dram_parameter("bo", [D], F32, isOutput=False)
    B1 = nc.declare_dram_parameter("b1", [DFF], F32, isOutput=False)
    B2 = nc.declare_dram_parameter("b2", [D], F32, isOutput=False)
    Y = nc.declare_dram_parameter("y", [QT, D], F32, isOutput=True)

    with TileContext(nc) as tc:
        with (
            tc.tile_pool(name="const", bufs=1) as cpool,
            tc.tile_pool(name="dram", bufs=1, space="DRAM") as dpool,
            tc.tile_pool(name="x2p", bufs=1) as x2p,
        ):
            ident32 = cpool.tile([128, 128], F32)
            make_identity(nc, ident32)
            ident16 = cpool.tile([128, 128], BF16)
            nc.vector.tensor_copy(ident16, ident32)
            eps = cpool.tile([128, 1], F32)
            nc.vector.memset(eps, EPS)
            ones64 = cpool.tile([1, 64], BF16)
            nc.vector.memset(ones64, 1.0)
            ones128f = cpool.tile([1, 128], F32)
            nc.vector.memset(ones128f, 1.0)
            ones128 = cpool.tile([1, 128], F32R)
            nc.vector.tensor_copy(ones128, ones128f)
            bqT = cpool.tile([128, 8], F32)
            nc.sync.dma_start(out=bqT, in_=BQ[:].rearrange("(t p) -> p t", p=128))
            bkT = cpool.tile([128, 8], F32)
            nc.sync.dma_start(out=bkT, in_=BK[:].rearrange("(t p) -> p t", p=128))
            b1T = cpool.tile([128, 32], F32)
            nc.sync.dma_start(out=b1T, in_=B1[:].rearrange("(t p) -> p t", p=128))
            bv_bc = cpool.tile([128, D], F32)
            nc.sync.dma_start(out=bv_bc, in_=BV[:].partition_broadcast(128))
            bo_bc = cpool.tile([128, D], F32)
            nc.sync.dma_start(out=bo_bc, in_=BO[:].partition_broadcast(128))
            b2_bc = cpool.tile([128, D], F32)
            nc.sync.dma_start(out=b2_bc, in_=B2[:].partition_broadcast(128))

            # DRAM bounce buffers for the K/V AllGathers (flat, partition-major)
            kT_loc = dpool.tile([128, 8 * QT], FP8)
            v_loc = dpool.tile([128, 4 * H * 65], FP8)
            kT_full = dpool.tile([4, 128, 8 * QT], FP8)
            v_full = dpool.tile([4, 128, 4 * H * 65], FP8)

            x2 = x2p.tile([128, 4, D], F32)     # post-attention residual
            attn128 = x2p.tile([128, 8, QT], BF16)  # normalized attn^T

            # ---- Phase 1: LN1 + K/V proj (own tokens) + gathers + Q proj ----
            with tc.tile_pool(name="qp", bufs=1) as qp:
              with (
                tc.tile_pool(name="kvloc", bufs=1) as kvp,
                tc.tile_pool(name="lnp", bufs=3) as lnp,
                tc.tile_pool(name="hTp", bufs=1) as hTp,
                tc.tile_pool(name="wsm", bufs=6) as wsm,
                tc.tile_pool(name="bcp", bufs=1) as bcp,
                tc.tile_pool(name="psT", bufs=2, space="PSUM") as psT,
                tc.tile_pool(name="psS1", bufs=1, space="PSUM") as psS1,
                tc.tile_pool(name="psK", bufs=3, space="PSUM") as psK,
              ):
                # -- LN1: stats in token space, apply in transposed space --
                hqT = hTp.tile([128, 8, QT], BF16)
                mr_row = lnp.tile([1, 512], F32R, tag="mr_row")
                rs_row = lnp.tile([1, 512], F32R, tag="rs_row")
                for st in range(4):
                    xt_a = lnp.tile([128, 512], F32, tag="ln_xa")
                    nc.sync.dma_start(
                        out=xt_a, in_=XQ[st * 128:(st + 1) * 128, 0:512])
                    xt_b = lnp.tile([128, 512], F32, tag="ln_xb")
                    nc.sync.dma_start(
                        out=xt_b, in_=XQ[st * 128:(st + 1) * 128, 512:1024])
                    mr = _ln_stats(nc, lnp, eps, xt_a, xt_b)
                    pst = psT.tile([128, 128], F32, tag="tp")
                    nc.tensor.transpose(pst[0:1, :], mr[:, 0:1].bitcast(F32),
                                        ident32)
                    nc.vector.tensor_copy(
                        mr_row[:, st * 128:(st + 1) * 128], pst[0:1, :])
                    pst2 = psT.tile([128, 128], F32, tag="tp")
                    nc.tensor.transpose(pst2[0:1, :], mr[:, 1:2].bitcast(F32),
                                        ident32)
                    nc.vector.tensor_copy(
                        rs_row[:, st * 128:(st + 1) * 128], pst2[0:1, :])
                # broadcast across partitions via rank-1 matmuls (f32r)
                mr_ps = psS1.tile([128, 512], F32, tag="psS")
                nc.tensor.matmul(mr_ps, ones128, mr_row, start=True, stop=True)
                mr_bc = bcp.tile([128, 512], F32, tag="mr")
                nc.vector.tensor_copy(mr_bc, mr_ps)
                rs_ps = psS1.tile([128, 512], F32, tag="psS")
                nc.tensor.matmul(rs_ps, ones128, rs_row, start=True, stop=True)
                rs_bc = bcp.tile([128, 512], F32, tag="rs")
                nc.vector.tensor_copy(rs_bc, rs_ps)
                for dt in range(8):
                    xtt = lnp.tile([128, 512], F32, tag="ln_xt")
                    nc.sync.dma_start(
                        out=xtt, in_=XQT[dt * 128:(dt + 1) * 128, :])
                    nc.vector.tensor_mul(xtt, xtt, rs_bc)
                    nc.vector.tensor_add(hqT[:, dt, :], xtt, mr_bc)

                # -- K proj -> feature-major [d, t] + bias, then gather --
                kloc_sb = kvp.tile([128, 8, QT], FP8)
                for ht in range(8):
                    wcol = wsm.tile([128, 8, 128], BF16, tag="w")
                    nc.sync.dma_start(out=wcol, in_=WKR[ht])
                    psk = psK.tile([128, 512], F32, tag="psK")
                    for dt in range(8):
                        nc.tensor.matmul(
                            psk, wcol[:, dt, :], hqT[:, dt, :],
                            start=(dt == 0), stop=(dt == 7),
                        )
                    nc.vector.tensor_scalar_add(
                        kloc_sb[:, ht, :], psk, bkT[:, ht:ht + 1]
                    )
                nc.sync.dma_start(
                    out=kT_loc[:, :],
                    in_=kloc_sb[:].rearrange("p t n -> p (t n)"),
                )
                nc.gpsimd.collective_compute(
                    "AllGather", ALU.bypass, replica_groups=RG,
                    ins=[kT_loc.opt()], outs=[kT_full.opt()],
                )

                # -- V proj -> token-major [t, (h 65)] + bias + ones col --
                vloc_sb = kvp.tile([128, 4, H, 65], FP8)
                nc.vector.memset(vloc_sb[:, :, :, 64:65], 1.0)
                for hc in range(2):
                    wv_sb = wsm.tile([128, 8, 512], BF16, tag="wv",
                                     name=f"wv{hc}")
                    for dq in range(4):
                        nc.sync.dma_start(
                            out=wv_sb[:, dq * 2:(dq + 1) * 2, :],
                            in_=WVR[hc, :, dq * 2:(dq + 1) * 2, :],
                        )
                    for st in range(4):
                        psv = psK.tile([128, 512], F32, tag="psK")
                        for dt in range(8):
                            nc.tensor.matmul(
                                psv,
                                hqT[:, dt, st * 128:(st + 1) * 128],
                                wv_sb[:, dt, :],
                                start=(dt == 0), stop=(dt == 7),
                            )
                        nc.vector.tensor_add(
                            vloc_sb[:, st, hc * 8:(hc + 1) * 8, 0:64],
                            psv.rearrange("p (h d) -> p h d", h=8),
                            bv_bc[:, hc * 512:(hc + 1) * 512].rearrange(
                                "p (h d) -> p h d", h=8),
                        )
                nc.scalar.dma_start(
                    out=v_loc[:, :],
                    in_=vloc_sb[:].rearrange("p s h d -> p (s h d)"),
                )
                nc.gpsimd.collective_compute(
                    "AllGather", ALU.bypass, replica_groups=RG,
                    ins=[v_loc.opt()], outs=[v_full.opt()],
                )

                # -- Q proj (overlaps with the gathers) --
                Q_sb = qp.tile([128, 8, QT], FP8)
                for ht in range(8):
                    wcol = wsm.tile([128, 8, 128], BF16, tag="w")
                    nc.sync.dma_start(out=wcol, in_=WQR[ht])
                    psq = psK.tile([128, 512], F32, tag="psK")
                    for dt in range(8):
                        nc.tensor.matmul(
                            psq, wcol[:, dt, :], hqT[:, dt, :],
                            start=(dt == 0), stop=(dt == 7),
                        )
                    nc.vector.tensor_scalar_add(
                        Q_sb[:, ht, :], psq, bqT[:, ht:ht + 1]
                    )

              # ---- Phase 2+3: load gathered K/V, pipelined attention ----
              with (
                  tc.tile_pool(name="kvall", bufs=1) as kva,
                  tc.tile_pool(name="pp", bufs=36) as ppl,
                  tc.tile_pool(name="accw", bufs=2) as accwp,
                  tc.tile_pool(name="dnp", bufs=2) as dnp,
                  tc.tile_pool(name="psS", bufs=4, space="PSUM") as psS,
                  tc.tile_pool(name="psA", bufs=4, space="PSUM") as psA,
              ):
                  KT_all = kva.tile([128, 4, 8, QT], FP8)
                  V_all = kva.tile([128, 4, 4, H, 65], FP8)
                  for g in range(4):
                      for half in range(2):
                          nc.sync.dma_start(
                              out=KT_all[:, g, half * 4:(half + 1) * 4, :],
                              in_=kT_full[g][:, half * 2048:(half + 1) * 2048]
                              .rearrange("p (t n) -> p t n", t=4),
                          )
                  for g in range(4):
                      for q in range(4):
                          nc.scalar.dma_start(
                              out=V_all[:, g, q, :, :],
                              in_=v_full[g][:, q * 1040:(q + 1) * 1040]
                              .rearrange("p (h d) -> p h d", h=H),
                          )

                  # per (wave, chunk) step: 16 score matmuls + 8 paired exps
                  # + 16 attn@V matmuls; scores run one step ahead; each
                  # wave's attention accumulators are normalized in-phase
                  steps = [(w, g) for w in range(4) for g in range(4)]
                  psa = {}
                  p_tiles = {}

                  def emit_scores(w, g):
                      for h in range(w * 4, w * 4 + 4):
                          kj, ko = h // 2, (h % 2) * 64
                          for ktl in range(4):
                              pss = psS.tile([128, 512], F32, tag="psS",
                                             name=f"pss{h}_{g}_{ktl}")
                              nc.tensor.matmul(
                                  pss,
                                  KT_all[ko:ko + 64, g, kj,
                                         ktl * 128:(ktl + 1) * 128],
                                  Q_sb[ko:ko + 64, kj, :],
                                  start=True, stop=True,
                              )
                              P = ppl.tile([128, 512], FP8, tag="P",
                                           name=f"P{h}_{g}_{ktl}")
                              nc.scalar.activation(P, pss, AF.Exp,
                                                   scale=0.125)
                              p_tiles[(h, g, ktl)] = P

                  def emit_attnv(w, g):
                      for h in range(w * 4, w * 4 + 4):
                          if g == 0:
                              psa[h] = psA.tile([65, 512], F32, tag="psA",
                                                name=f"psa{h}")
                          for ktl in range(4):
                              nc.tensor.matmul(
                                  psa[h],
                                  V_all[:, g, ktl, h, :],
                                  p_tiles.pop((h, g, ktl)),
                                  start=(g == 0 and ktl == 0),
                                  stop=(g == 3 and ktl == 3),
                              )

                  def emit_normalize(w):
                      heads = list(range(w * 4, w * 4 + 4))
                      aw = accwp.tile([65, 4, 512], BF16, tag="aw",
                                      name=f"aw{w}")
                      for i, h in enumerate(heads):
                          nc.vector.tensor_copy(aw[:, i, :], psa.pop(h))
                      dn = dnp.tile([4, 512], BF16, tag="dn", name=f"dn{w}")
                      for i in range(4):
                          nc.sync.dma_start(out=dn[i:i + 1, :],
                                            in_=aw[64:65, i, :])
                      rw = dnp.tile([4, 512], F32, tag="rw", name=f"rw{w}")
                      nc.vector.reciprocal(rw, dn)
                      rwb = dnp.tile([4, 512], BF16, tag="rwb", name=f"rwb{w}")
                      nc.vector.tensor_copy(rwb, rw)
                      rrow = dnp.tile([1, 4, 512], BF16, tag="rrow",
                                      name=f"rrow{w}")
                      for i in range(4):
                          nc.sync.dma_start(out=rrow[0:1, i, :],
                                            in_=rwb[i:i + 1, :])
                      for i, h in enumerate(heads):
                          rbt = psS.tile([128, 512], F32, tag="psS",
                                         name=f"rb{w}_{i}")
                          nc.tensor.matmul(rbt[0:64, :], ones64,
                                           rrow[0:1, i, :],
                                           start=True, stop=True)
                          kj, ko = h // 2, (h % 2) * 64
                          nc.vector.tensor_mul(
                              attn128[ko:ko + 64, kj, :], aw[0:64, i, :],
                              rbt[0:64, :]
                          )

                  emit_scores(*steps[0])
                  for i in range(len(steps)):
                      if i + 1 < len(steps):
                          emit_scores(*steps[i + 1])
                      emit_attnv(*steps[i])
                      if steps[i][1] == 3:
                          emit_normalize(steps[i][0])

            # ---- Phase 4: O proj + residual ----
            with (
                tc.tile_pool(name="xqp", bufs=1) as xqp,
                tc.tile_pool(name="dwo", bufs=8) as dwo,
                tc.tile_pool(name="dtmp", bufs=4) as dtmp,
                tc.tile_pool(name="psO", bufs=4, space="PSUM") as psO,
            ):
                xq_sb = xqp.tile([128, 4, D], F32)
                for st in range(4):
                    nc.sync.dma_start(
                        out=xq_sb[:, st, :],
                        in_=XQ[st * 128:(st + 1) * 128, :])
                for c in range(2):
                    po = [psO.tile([128, 512], F32, tag="psO",
                                   name=f"po{c}_{i}") for i in range(4)]
                    for j in range(8):
                        wot = dwo.tile([128, 512], BF16, tag="wo")
                        nc.sync.dma_start(
                            out=wot,
                            in_=WO[j * 128:(j + 1) * 128,
                                   c * 512:(c + 1) * 512],
                        )
                        for qt in range(4):
                            nc.tensor.matmul(
                                po[qt], attn128[:, j, qt * 128:(qt + 1) * 128],
                                wot, start=(j == 0), stop=(j == 7),
                            )
                    for qt in range(4):
                        t1 = dtmp.tile([128, 512], F32, tag="t1")
                        nc.vector.tensor_add(
                            t1, po[qt], bo_bc[:, c * 512:(c + 1) * 512]
                        )
                        nc.vector.tensor_add(
                            x2[:, qt, c * 512:(c + 1) * 512],
                            t1,
                            xq_sb[:, qt, c * 512:(c + 1) * 512],
                        )

            # ---- Phase 5: LN2 + MLP + residual ----
            with (
                tc.tile_pool(name="lnp2", bufs=3) as lnp2,
                tc.tile_pool(name="h2p", bufs=1) as h2p,
                tc.tile_pool(name="gp", bufs=1) as gp,
                tc.tile_pool(name="wfp", bufs=8) as wfp,
                tc.tile_pool(name="w2p", bufs=8) as w2p,
                tc.tile_pool(name="yp", bufs=2) as yp,
            ):
                h2T = h2p.tile([128, 8, QT], FP8)
                G = gp.tile([128, 32, QT], FP8)
                with (
                    tc.tile_pool(name="psT2", bufs=2, space="PSUM") as psT2,
                    tc.tile_pool(name="psF", bufs=4, space="PSUM") as psF,
                ):
                    # LN2 from SBUF-resident x2, transposed output
                    for st in range(4):
                        mr = _ln_stats(nc, lnp2, eps,
                                       x2[:, st, 0:512], x2[:, st, 512:1024])
                        h2 = lnp2.tile([128, D], BF16, tag="ln_h")
                        # h2 = x2*rstd + (-mu*rstd)
                        nc.vector.tensor_scalar(
                            h2, x2[:, st, :], mr[:, 1:2].bitcast(F32),
                            mr[:, 0:1].bitcast(F32), ALU.mult, ALU.add
                        )
                        for dt in range(8):
                            pst = psT2.tile([128, 128], BF16, tag="tp2")
                            nc.tensor.transpose(
                                pst, h2[:, dt * 128:(dt + 1) * 128], ident16
                            )
                            nc.vector.tensor_copy(
                                h2T[:, dt, st * 128:(st + 1) * 128], pst
                            )

                    # MLP1: gelu(h2 @ w1 + b1), transposed output [dff, q]
                    for ft in range(32):
                        w1c = wfp.tile([128, 8, 128], FP8, tag="w1")
                        nc.sync.dma_start(out=w1c, in_=W1R[ft])
                        psf = psF.tile([128, 512], F32, tag="psF")
                        for j in range(4):
                            nc.tensor.matmul(
                                psf, w1c[:, 2 * j:2 * j + 2, :],
                                h2T[:, 2 * j:2 * j + 2, :],
                                start=(j == 0), stop=(j == 3),
                                perf_mode=PM.DoubleRow,
                            )
                        nc.scalar.activation(
                            G[:, ft, :], psf, AF.Gelu, bias=b1T[:, ft:ft + 1]
                        )

                # MLP2: y = G^T @ w2 + b2 + x2
                with tc.tile_pool(name="psY", bufs=4, space="PSUM") as psY:
                    for c in range(2):
                        py = [psY.tile([128, 512], F32, tag="psY",
                                       name=f"py{c}_{i}") for i in range(4)]
                        for jp in range(16):
                            w2t = w2p.tile([128, 2, 512], FP8, tag="w2")
                            nc.sync.dma_start(
                                out=w2t,
                                in_=W2R[jp, :, :, c * 512:(c + 1) * 512],
                            )
                            for qt in range(4):
                                nc.tensor.matmul(
                                    py[qt],
                                    G[:, 2 * jp:2 * jp + 2,
                                      qt * 128:(qt + 1) * 128],
                                    w2t, start=(jp == 0), stop=(jp == 15),
                                    perf_mode=PM.DoubleRow,
                                )
                        for qt in range(4):
                            t1 = yp.tile([128, 512], F32, tag="yt1")
                            nc.vector.tensor_add(
                                t1, py[qt], b2_bc[:, c * 512:(c + 1) * 512]
                            )
                            yt = yp.tile([128, 512], F32, tag="yt2")
                            nc.vector.tensor_add(
                                yt, t1, x2[:, qt, c * 512:(c + 1) * 512]
                            )
                            nc.sync.dma_start(
                                out=Y[qt * 128:(qt + 1) * 128,
                                      c * 512:(c + 1) * 512],
                                in_=yt,
                            )

    nc.compile()
    return nc


_NC = None


def _get_nc():
    global _NC
    if _NC is None:
        _NC = _build()
    return _NC


def make_in_maps(inputs):
    f32 = lambda a: np.ascontiguousarray(np.asarray(a, dtype=np.float32))
    bf16 = lambda a: np.ascontiguousarray(
        np.asarray(a, dtype=np.float32).astype(ml_dtypes.bfloat16)
    )
    fp8 = lambda a: np.ascontiguousarray(
        np.asarray(a, dtype=np.float32).astype(ml_dtypes.float8_e4m3fn)
    )
    x = f32(inputs["x"])
    g1, b1l = f32(inputs["ln1_g"]), f32(inputs["ln1_b"])
    g2, b2l = f32(inputs["ln2_g"]), f32(inputs["ln2_b"])
    wq, wk, wv = f32(inputs["wq"]), f32(inputs["wk"]), f32(inputs["wv"])
    w1 = f32(inputs["w1"])

    # Fold LayerNorm affine params into the following projections (exact).
    wq_e = g1[:, None] * wq
    wk_e = g1[:, None] * wk
    wv_e = g1[:, None] * wv
    w1_e = g2[:, None] * w1

    # host pre-layouts: [out-block, partition, dt, n]
    def col_blocks(w, nblk, nsz):
        # w [1024, nblk*nsz] -> [nblk, 128, 8, nsz]
        return np.ascontiguousarray(
            w.reshape(8, 128, nblk, nsz).transpose(2, 1, 0, 3))

    common = {
        "wqr": bf16(col_blocks(wq_e, 8, 128)),
        "wkr": bf16(col_blocks(wk_e, 8, 128)),
        "wvr": bf16(col_blocks(wv_e, 2, 512)),
        "wo": bf16(inputs["wo"]),
        "w1r": fp8(col_blocks(w1_e, 32, 128)),
        "w2r": fp8(np.asarray(inputs["w2"], dtype=np.float32)
           .reshape(16, 2, 128, D).transpose(0, 2, 1, 3)),
        "bq": f32(inputs["bq"] + b1l @ wq),
        "bk": f32(inputs["bk"] + b1l @ wk),
        "bv": f32(inputs["bv"] + b1l @ wv),
        "bo": f32(inputs["bo"]),
        "b1": f32(inputs["b1"] + b2l @ w1),
        "b2": f32(inputs["b2"]),
    }
    in_maps = []
    for c in range(NCORES):
        b = c // 4
        qoff = (c % 4) * QT
        m = dict(common)
        m["xq"] = np.ascontiguousarray(x[b, qoff:qoff + QT])
        m["xqt"] = np.ascontiguousarray(x[b, qoff:qoff + QT].T)
        in_maps.append(m)
    return in_maps


def kernel(x, ln1_g, ln1_b, wq, bq, wk, bk, wv, bv, wo, bo, w1, b1, w2, b2,
           ln2_g, ln2_b):
    inputs = {
        "x": x, "ln1_g": ln1_g, "ln1_b": ln1_b,
        "wq": wq, "bq": bq, "wk": wk, "bk": bk, "wv": wv, "bv": bv,
        "wo": wo, "bo": bo, "w1": w1, "b1": b1, "w2": w2, "b2": b2,
        "ln2_g": ln2_g, "ln2_b": ln2_b,
    }
    in_maps = make_in_maps(inputs)
    nc = _get_nc()
    res = run_bass_kernel_spmd(nc, in_maps, core_ids=list(range(NCORES)))

    y = np.empty((B, S, D), dtype=np.float32)
    for c in range(NCORES):
        b = c // 4
        qoff = (c % 4) * QT
        y[b, qoff:qoff + QT] = res.results[c]["y"]
    return y


# revision 24
# speedup vs baseline: 1.0320x; 1.0320x over previous
"""Transformer encoder layer (LN -> MHA -> residual -> LN -> MLP -> residual)
on 8 Trainium2 NeuronCores.

Sharding: token-parallel over the 4096 (batch*seq) tokens, 512 query-tokens
per core.  Each core computes K/V projections only for its OWN 512 tokens;
the full 2048-token K/V per batch is assembled with two AllGather
collectives (bf16, ~1 MB each) across the 4-core group sharing a batch.
Collectives run on TOPSP/SDMA and overlap with the Q projection.

On-chip layout: activations are kept feature-major ("transposed", [d, token])
so every matmul contracts along the partition dim.  Weights are pre-arranged
on the host so every weight DMA is one contiguous run per partition (DMA
descriptor count is the latency driver, not bytes).  Matmul operands are
bf16; accumulation stays fp32 in PSUM.  Softmax is computed unnormalized
(scores are bounded so plain exp is safe and algebraically identical); the
denominator comes from a ones-column interleaved into V before the gather,
and each head's attention accumulator stays resident in one PSUM bank
across all 16 k-tiles.  The score matmuls run one (wave, chunk) step ahead
of the attn@V matmuls so the exp's on the scalar engine pipeline behind
full-speed PE bursts.

LayerNorm gains/biases are folded into the following projections on the host
(exact algebra: (g*xhat+b) @ W = xhat @ (diag(g) W) + b @ W).
"""

import numpy as np
import ml_dtypes

import concourse.bass as bass
import concourse.mybir as mybir
from concourse import bacc
from concourse.tile import TileContext
from concourse.bass_utils import run_bass_kernel_spmd
from concourse.masks import make_identity

F32 = mybir.dt.float32
F32R = mybir.dt.float32r
BF16 = mybir.dt.bfloat16
FP8 = mybir.dt.float8e4
PM = mybir.MatmulPerfMode
AF = mybir.ActivationFunctionType
ALU = mybir.AluOpType

B, S, D = 2, 2048, 1024
H, HD = 16, 64
DFF = 4 * D
NCORES = 8
QT = 512            # query tokens per core
EPS = 1e-5
RG = [[0, 1, 2, 3], [4, 5, 6, 7]]  # replica groups (one per batch)


def _ln_stats(nc, lnp, eps, xt_a, xt_b):
    """bn stats over two [128, 512] token half-tiles -> (-mu*rstd, rstd)."""
    stats = lnp.tile([128, 2, 6], F32, tag="ln_st")
    nc.vector.bn_stats(stats[:, 0, :], xt_a)
    nc.vector.bn_stats(stats[:, 1, :], xt_b)
    mv = lnp.tile([128, 2], F32, tag="ln_mv")
    nc.vector.bn_aggr(mv, stats)
    sd = lnp.tile([128, 1], F32, tag="ln_sd")
    nc.scalar.activation(sd, mv[:, 1:2], AF.Sqrt, bias=eps[:, 0:1])
    rstd = lnp.tile([128, 1], F32, tag="ln_rs")
    nc.vector.reciprocal(rstd, sd)
    mr = lnp.tile([128, 2], F32R, tag="ln_mr")
    nc.vector.tensor_scalar(
        mr[:, 0:1], mv[:, 0:1], rstd, -1.0, ALU.mult, ALU.mult
    )
    nc.vector.tensor_copy(mr[:, 1:2], rstd)
    return mr


def _build():
    nc = bacc.Bacc(None, target_bir_lowering=False, num_devices=NCORES)

    XQ = nc.declare_dram_parameter("xq", [QT, D], F32, isOutput=False)
    XQT = nc.declare_dram_parameter("xqt", [D, QT], F32, isOutput=False)
    # host-prearranged weights: one contiguous run per partition per load
    WQR = nc.declare_dram_parameter("wqr", [8, 128, 8, 128], BF16, isOutput=False)
    WKR = nc.declare_dram_parameter("wkr", [8, 128, 8, 128], BF16, isOutput=False)
    WVR = nc.declare_dram_parameter("wvr", [2, 128, 8, 512], BF16, isOutput=False)
    WO = nc.declare_dram_parameter("wo", [D, D], BF16, isOutput=False)
    W1R = nc.declare_dram_parameter("w1r", [32, 128, 8, 128], FP8, isOutput=False)
    W2R = nc.declare_dram_parameter("w2r", [16, 128, 2, D], FP8, isOutput=False)
    BQ = nc.declare_dram_parameter("bq", [D], F32, isOutput=False)
    BK = nc.declare_dram_parameter("bk", [D], F32, isOutput=False)
    BV = nc.declare_dram_parameter("bv", [D], F32, isOutput=False)
    BO = nc.declare_dram_parameter("bo", [D], F32, isOutput=False)
    B1 = nc.declare_dram_parameter("b1", [DFF], F32, isOutput=False)
    B2 = nc.declare_dram_parameter("b2", [D], F32, isOutput=False)
    Y = nc.declare_dram_parameter("y", [QT, D], F32, isOutput=True)

    with TileContext(nc) as tc:
        with (
            tc.tile_pool(name="const", bufs=1) as cpool,
            tc.tile_pool(name="dram", bufs=1, space="DRAM") as dpool,
            tc.tile_pool(name="x2p", bufs=1) as x2p,
        ):
            ident32 = cpool.tile([128, 128], F32)
            make_identity(nc, ident32)
            ident16 = cpool.tile([128, 128], BF16)
            nc.vector.tensor_copy(ident16, ident32)
            eps = cpool.tile([128, 1], F32)
            nc.vector.memset(eps, EPS)
            ones64 = cpool.tile([1, 64], BF16)
            nc.vector.memset(ones64, 1.0)
            ones128f = cpool.tile([1, 128], F32)
            nc.vector.memset(ones128f, 1.0)
            ones128 = cpool.tile([1, 128], F32R)
            nc.vector.tensor_copy(ones128, ones128f)
            bqT = cpool.tile([128, 8], F32)
            nc.sync.dma_start(out=bqT, in_=BQ[:].rearrange("(t p) -> p t", p=128))
            bkT = cpool.tile([128, 8], F32)
            nc.sync.dma_start(out=bkT, in_=BK[:].rearrange("(t p) -> p t", p=128))
            b1T = cpool.tile([128, 32], F32)
            nc.sync.dma_start(out=b1T, in_=B1[:].rearrange("(t p) -> p t", p=128))
            bv_bc = cpool.tile([128, D], F32)
            nc.sync.dma_start(out=bv_bc, in_=BV[:].partition_broadcast(128))
            bo_bc = cpool.tile([128, D], F32)
            nc.sync.dma_start(out=bo_bc, in_=BO[:].partition_broadcast(128))
            b2_bc = cpool.tile([128, D], F32)
            nc.sync.dma_start(out=b2_bc, in_=B2[:].partition_broadcast(128))

            # DRAM bounce buffers for the K/V AllGathers (flat, partition-major)
            kT_loc = dpool.tile([128, 8 * QT], FP8)
            v_loc = dpool.tile([128, 4 * H * 65], FP8)
            kT_full = dpool.tile([4, 128, 8 * QT], FP8)
            v_full = dpool.tile([4, 128, 4 * H * 65], FP8)

            x2 = x2p.tile([128, 4, D], F32)     # post-attention residual
            attn128 = x2p.tile([128, 8, QT], BF16)  # normalized attn^T

            # ---- Phase 1: LN1 + K/V proj (own tokens) + gathers + Q proj ----
            with tc.tile_pool(name="qp", bufs=1) as qp:
              with (
                tc.tile_pool(name="kvloc", bufs=1) as kvp,
                tc.tile_pool(name="lnp", bufs=3) as lnp,
                tc.tile_pool(name="hTp", bufs=1) as hTp,
                tc.tile_pool(name="wsm", bufs=6) as wsm,
                tc.tile_pool(name="bcp", bufs=1) as bcp,
                tc.tile_pool(name="psT", bufs=2, space="PSUM") as psT,
                tc.tile_pool(name="psS1", bufs=1, space="PSUM") as psS1,
                tc.tile_pool(name="psK", bufs=3, space="PSUM") as psK,
              ):
                # -- LN1: stats in token space, apply in transposed space --
                hqT = hTp.tile([128, 8, QT], BF16)
                mr_row = lnp.tile([1, 512], F32R, tag="mr_row")
                rs_row = lnp.tile([1, 512], F32R, tag="rs_row")
                for st in range(4):
                    xt_a = lnp.tile([128, 512], F32, tag="ln_xa")
                    nc.sync.dma_start(
                        out=xt_a, in_=XQ[st * 128:(st + 1) * 128, 0:512])
                    xt_b = lnp.tile([128, 512], F32, tag="ln_xb")
                    nc.sync.dma_start(
                        out=xt_b, in_=XQ[st * 128:(st + 1) * 128, 512:1024])
                    mr = _ln_stats(nc, lnp, eps, xt_a, xt_b)
                    pst = psT.tile([128, 128], F32, tag="tp")
                    nc.tensor.transpose(pst[0:1, :], mr[:, 0:1].bitcast(F32),
                                        ident32)
                    nc.vector.tensor_copy(
                        mr_row[:, st * 128:(st + 1) * 128], pst[0:1, :])
                    pst2 = psT.tile([128, 128], F32, tag="tp")
                    nc.tensor.transpose(pst2[0:1, :], mr[:, 1:2].bitcast(F32),
                                        ident32)
                    nc.vector.tensor_copy(
                        rs_row[:, st * 128:(st + 1) * 128], pst2[0:1, :])
                # broadcast across partitions via rank-1 matmuls (f32r)
                mr_ps = psS1.tile([128, 512], F32, tag="psS")
                nc.tensor.matmul(mr_ps, ones128, mr_row, start=True, stop=True)
                mr_bc = bcp.tile([128, 512], F32, tag="mr")
                nc.vector.tensor_copy(mr_bc, mr_ps)
                rs_ps = psS1.tile([128, 512], F32, tag="psS")
                nc.tensor.matmul(rs_ps, ones128, rs_row, start=True, stop=True)
                rs_bc = bcp.tile([128, 512], F32, tag="rs")
                nc.vector.tensor_copy(rs_bc, rs_ps)
                for dt in range(8):
                    xtt = lnp.tile([128, 512], F32, tag="ln_xt")
                    nc.sync.dma_start(
                        out=xtt, in_=XQT[dt * 128:(dt + 1) * 128, :])
                    nc.vector.tensor_mul(xtt, xtt, rs_bc)
                    nc.vector.tensor_add(hqT[:, dt, :], xtt, mr_bc)

                # -- K proj -> feature-major [d, t] + bias, then gather --
                kloc_sb = kvp.tile([128, 8, QT], FP8)
                for ht in range(8):
                    wcol = wsm.tile([128, 8, 128], BF16, tag="w")
                    nc.sync.dma_start(out=wcol, in_=WKR[ht])
                    psk = psK.tile([128, 512], F32, tag="psK")
                    for dt in range(8):
                        nc.tensor.matmul(
                            psk, wcol[:, dt, :], hqT[:, dt, :],
                            start=(dt == 0), stop=(dt == 7),
                        )
                    nc.vector.tensor_scalar_add(
                        kloc_sb[:, ht, :], psk, bkT[:, ht:ht + 1]
                    )
                nc.sync.dma_start(
                    out=kT_loc[:, :],
                    in_=kloc_sb[:].rearrange("p t n -> p (t n)"),
                )
                nc.gpsimd.collective_compute(
                    "AllGather", ALU.bypass, replica_groups=RG,
                    ins=[kT_loc.opt()], outs=[kT_full.opt()],
                )

                # -- V proj -> token-major [t, (h 65)] + bias + ones col --
                vloc_sb = kvp.tile([128, 4, H, 65], FP8)
                nc.vector.memset(vloc_sb[:, :, :, 64:65], 1.0)
                for hc in range(2):
                    wv_sb = wsm.tile([128, 8, 512], BF16, tag="wv",
                                     name=f"wv{hc}")
                    for dq in range(4):
                        nc.sync.dma_start(
                            out=wv_sb[:, dq * 2:(dq + 1) * 2, :],
                            in_=WVR[hc, :, dq * 2:(dq + 1) * 2, :],
                        )
                    for st in range(4):
                        psv = psK.tile([128, 512], F32, tag="psK")
                        for dt in range(8):
                            nc.tensor.matmul(
                                psv,
                                hqT[:, dt, st * 128:(st + 1) * 128],
                                wv_sb[:, dt, :],
                                start=(dt == 0), stop=(dt == 7),
                            )
                        nc.vector.tensor_add(
                            vloc_sb[:, st, hc * 8:(hc + 1) * 8, 0:64],
                            psv.rearrange("p (h d) -> p h d", h=8),
                            bv_bc[:, hc * 512:(hc + 1) * 512].rearrange(
                                "p (h d) -> p h d", h=8),
                        )
                nc.scalar.dma_start(
                    out=v_loc[:, :],
                    in_=vloc_sb[:].rearrange("p s h d -> p (s h d)"),
                )
                nc.gpsimd.collective_compute(
                    "AllGather", ALU.bypass, replica_groups=RG,
                    ins=[v_loc.opt()], outs=[v_full.opt()],
                )

                # -- Q proj (overlaps with the gathers) --
                Q_sb = qp.tile([128, 8, QT], FP8)
                for ht in range(8):
                    wcol = wsm.tile([128, 8, 128], BF16, tag="w")
                    nc.sync.dma_start(out=wcol, in_=WQR[ht])
                    psq = psK.tile([128, 512], F32, tag="psK")
                    for dt in range(8):
                        nc.tensor.matmul(
                            psq, wcol[:, dt, :], hqT[:, dt, :],
                            start=(dt == 0), stop=(dt == 7),
                        )
                    nc.vector.tensor_scalar_add(
                        Q_sb[:, ht, :], psq, bqT[:, ht:ht + 1]
                    )

              # ---- Phase 2+3: load gathered K/V, pipelined attention ----
              with (
                  tc.tile_pool(name="kvall", bufs=1) as kva,
                  tc.tile_pool(name="pp", bufs=20) as ppl,
                  tc.tile_pool(name="accw", bufs=2) as accwp,
                  tc.tile_pool(name="dnp", bufs=2) as dnp,
                  tc.tile_pool(name="psS", bufs=2, space="PSUM") as psS,
                  tc.tile_pool(name="psA", bufs=4, space="PSUM") as psA,
              ):
                  KT_all = kva.tile([128, 4, 8, QT], FP8)
                  V_all = kva.tile([128, 4, 4, H, 65], FP8)
                  for g in range(4):
                      for half in range(2):
                          nc.sync.dma_start(
                              out=KT_all[:, g, half * 4:(half + 1) * 4, :],
                              in_=kT_full[g][:, half * 2048:(half + 1) * 2048]
                              .rearrange("p (t n) -> p t n", t=4),
                          )
                  for g in range(4):
                      for q in range(4):
                          nc.scalar.dma_start(
                              out=V_all[:, g, q, :, :],
                              in_=v_full[g][:, q * 1040:(q + 1) * 1040]
                              .rearrange("p (h d) -> p h d", h=H),
                          )

                  # per (wave, chunk) step: 16 score matmuls + 8 paired exps
                  # + 16 attn@V matmuls; scores run one step ahead; each
                  # wave's attention accumulators are normalized in-phase
                  steps = [(w, g) for w in range(4) for g in range(4)]
                  psa = {}
                  p_tiles = {}

                  def emit_scores(w, g):
                      for h in range(w * 4, w * 4 + 4):
                          kj, ko = h // 2, (h % 2) * 64
                          for pair in range(2):
                              pss2 = psS.tile([128, 2, 512], F32, tag="psS",
                                              name=f"pss{h}_{g}_{pair}")
                              for j in range(2):
                                  ktl = pair * 2 + j
                                  nc.tensor.matmul(
                                      pss2[:, j, :],
                                      KT_all[ko:ko + 64, g, kj,
                                             ktl * 128:(ktl + 1) * 128],
                                      Q_sb[ko:ko + 64, kj, :],
                                      start=True, stop=True,
                                  )
                              P2 = ppl.tile([128, 2, 512], FP8, tag="P",
                                            name=f"P{h}_{g}_{pair}")
                              nc.scalar.activation(P2, pss2, AF.Exp,
                                                   scale=0.125)
                              p_tiles[(h, g, pair)] = P2

                  def emit_attnv(w, g):
                      for h in range(w * 4, w * 4 + 4):
                          if g == 0:
                              psa[h] = psA.tile([65, 512], F32, tag="psA",
                                                name=f"psa{h}")
                          for pair in range(2):
                              P2 = p_tiles.pop((h, g, pair))
                              for j in range(2):
                                  ktl = pair * 2 + j
                                  nc.tensor.matmul(
                                      psa[h],
                                      V_all[:, g, ktl, h, :],
                                      P2[:, j, :],
                                      start=(g == 0 and ktl == 0),
                                      stop=(g == 3 and ktl == 3),
                                  )

                  def emit_normalize(w):
                      heads = list(range(w * 4, w * 4 + 4))
                      aw = accwp.tile([65, 4, 512], BF16, tag="aw",
                                      name=f"aw{w}")
                      for i, h in enumerate(heads):
                          nc.vector.tensor_copy(aw[:, i, :], psa.pop(h))
                      dn = dnp.tile([4, 512], BF16, tag="dn", name=f"dn{w}")
                      for i in range(4):
                          nc.sync.dma_start(out=dn[i:i + 1, :],
                                            in_=aw[64:65, i, :])
                      rw = dnp.tile([4, 512], F32, tag="rw", name=f"rw{w}")
                      nc.vector.reciprocal(rw, dn)
                      rwb = dnp.tile([4, 512], BF16, tag="rwb", name=f"rwb{w}")
                      nc.vector.tensor_copy(rwb, rw)
                      rrow = dnp.tile([1, 4, 512], BF16, tag="rrow",
                                      name=f"rrow{w}")
                      for i in range(4):
                          nc.sync.dma_start(out=rrow[0:1, i, :],
                                            in_=rwb[i:i + 1, :])
                      for i, h in enumerate(heads):
                          rbt = psS.tile([128, 2, 512], F32, tag="psS",
                                         name=f"rb{w}_{i}")
                          nc.tensor.matmul(rbt[0:64, 0, :], ones64,
                                           rrow[0:1, i, :],
                                           start=True, stop=True)
                          kj, ko = h // 2, (h % 2) * 64
                          nc.vector.tensor_mul(
                              attn128[ko:ko + 64, kj, :], aw[0:64, i, :],
                              rbt[0:64, 0, :]
                          )

                  emit_scores(*steps[0])
                  for i in range(len(steps)):
                      if i + 1 < len(steps):
                          emit_scores(*steps[i + 1])
                      emit_attnv(*steps[i])
                      if steps[i][1] == 3:
                          emit_normalize(steps[i][0])

            # ---- Phase 4: O proj + residual ----
            with (
                tc.tile_pool(name="xqp", bufs=1) as xqp,
                tc.tile_pool(name="dwo", bufs=8) as dwo,
                tc.tile_pool(name="dtmp", bufs=4) as dtmp,
                tc.tile_pool(name="psO", bufs=4, space="PSUM") as psO,
            ):
                xq_sb = xqp.tile([128, 4, D], F32)
                for st in range(4):
                    nc.sync.dma_start(
                        out=xq_sb[:, st, :],
                        in_=XQ[st * 128:(st + 1) * 128, :])
                for c in range(2):
                    po = [psO.tile([128, 512], F32, tag="psO",
                                   name=f"po{c}_{i}") for i in range(4)]
                    for j in range(8):
                        wot = dwo.tile([128, 512], BF16, tag="wo")
                        nc.sync.dma_start(
                            out=wot,
                            in_=WO[j * 128:(j + 1) * 128,
                                   c * 512:(c + 1) * 512],
                        )
                        for qt in range(4):
                            nc.tensor.matmul(
                                po[qt], attn128[:, j, qt * 128:(qt + 1) * 128],
                                wot, start=(j == 0), stop=(j == 7),
                            )
                    for qt in range(4):
                        t1 = dtmp.tile([128, 512], F32, tag="t1")
                        nc.vector.tensor_add(
                            t1, po[qt], bo_bc[:, c * 512:(c + 1) * 512]
                        )
                        nc.vector.tensor_add(
                            x2[:, qt, c * 512:(c + 1) * 512],
                            t1,
                            xq_sb[:, qt, c * 512:(c + 1) * 512],
                        )

            # ---- Phase 5: LN2 + MLP + residual ----
            with (
                tc.tile_pool(name="lnp2", bufs=3) as lnp2,
                tc.tile_pool(name="h2p", bufs=1) as h2p,
                tc.tile_pool(name="gp", bufs=1) as gp,
                tc.tile_pool(name="wfp", bufs=8) as wfp,
                tc.tile_pool(name="w2p", bufs=8) as w2p,
                tc.tile_pool(name="yp", bufs=2) as yp,
            ):
                h2T = h2p.tile([128, 8, QT], FP8)
                G = gp.tile([128, 32, QT], FP8)
                with (
                    tc.tile_pool(name="psT2", bufs=2, space="PSUM") as psT2,
                    tc.tile_pool(name="psF", bufs=4, space="PSUM") as psF,
                ):
                    # LN2 from SBUF-resident x2, transposed output
                    for st in range(4):
                        mr = _ln_stats(nc, lnp2, eps,
                                       x2[:, st, 0:512], x2[:, st, 512:1024])
                        h2 = lnp2.tile([128, D], BF16, tag="ln_h")
                        # h2 = x2*rstd + (-mu*rstd)
                        nc.vector.tensor_scalar(
                            h2, x2[:, st, :], mr[:, 1:2].bitcast(F32),
                            mr[:, 0:1].bitcast(F32), ALU.mult, ALU.add
                        )
                        for dt in range(8):
                            pst = psT2.tile([128, 128], BF16, tag="tp2")
                            nc.tensor.transpose(
                                pst, h2[:, dt * 128:(dt + 1) * 128], ident16
                            )
                            nc.vector.tensor_copy(
                                h2T[:, dt, st * 128:(st + 1) * 128], pst
                            )

                    # MLP1: gelu(h2 @ w1 + b1), transposed output [dff, q]
                    for ft in range(32):
                        w1c = wfp.tile([128, 8, 128], FP8, tag="w1")
                        nc.sync.dma_start(out=w1c, in_=W1R[ft])
                        psf = psF.tile([128, 512], F32, tag="psF")
                        for j in range(4):
                            nc.tensor.matmul(
                                psf, w1c[:, 2 * j:2 * j + 2, :],
                                h2T[:, 2 * j:2 * j + 2, :],
                                start=(j == 0), stop=(j == 3),
                                perf_mode=PM.DoubleRow,
                            )
                        nc.scalar.activation(
                            G[:, ft, :], psf, AF.Gelu, bias=b1T[:, ft:ft + 1]
                        )

                # MLP2: y = G^T @ w2 + b2 + x2
                with tc.tile_pool(name="psY", bufs=4, space="PSUM") as psY:
                    for c in range(2):
                        py = [psY.tile([128, 512], F32, tag="psY",
                                       name=f"py{c}_{i}") for i in range(4)]
                        for jp in range(16):
                            w2t = w2p.tile([128, 2, 512], FP8, tag="w2")
                            nc.sync.dma_start(
                                out=w2t,
                                in_=W2R[jp, :, :, c * 512:(c + 1) * 512],
                            )
                            for qt in range(4):
                                nc.tensor.matmul(
                                    py[qt],
                                    G[:, 2 * jp:2 * jp + 2,
                                      qt * 128:(qt + 1) * 128],
                                    w2t, start=(jp == 0), stop=(jp == 15),
                                    perf_mode=PM.DoubleRow,
                                )
                        for qt in range(4):
                            t1 = yp.tile([128, 512], F32, tag="yt1")
                            nc.vector.tensor_add(
                                t1, py[qt], b2_bc[:, c * 512:(c + 1) * 512]
                            )
                            yt = yp.tile([128, 512], F32, tag="yt2")
                            nc.vector.tensor_add(
                                yt, t1, x2[:, qt, c * 512:(c + 1) * 512]
                            )
                            nc.sync.dma_start(
                                out=Y[qt * 128:(qt + 1) * 128,
                                      c * 512:(c + 1) * 512],
                                in_=yt,
                            )

    nc.compile()
    return nc


_NC = None


def _get_nc():
    global _NC
    if _NC is None:
        _NC = _build()
    return _NC


def make_in_maps(inputs):
    f32 = lambda a: np.ascontiguousarray(np.asarray(a, dtype=np.float32))
    bf16 = lambda a: np.ascontiguousarray(
        np.asarray(a, dtype=np.float32).astype(ml_dtypes.bfloat16)
    )
    fp8 = lambda a: np.ascontiguousarray(
        np.asarray(a, dtype=np.float32).astype(ml_dtypes.float8_e4m3fn)
    )
    x = f32(inputs["x"])
    g1, b1l = f32(inputs["ln1_g"]), f32(inputs["ln1_b"])
    g2, b2l = f32(inputs["ln2_g"]), f32(inputs["ln2_b"])
    wq, wk, wv = f32(inputs["wq"]), f32(inputs["wk"]), f32(inputs["wv"])
    w1 = f32(inputs["w1"])

    # Fold LayerNorm affine params into the following projections (exact).
    wq_e = g1[:, None] * wq
    wk_e = g1[:, None] * wk
    wv_e = g1[:, None] * wv
    w1_e = g2[:, None] * w1

    # host pre-layouts: [out-block, partition, dt, n]
    def col_blocks(w, nblk, nsz):
        # w [1024, nblk*nsz] -> [nblk, 128, 8, nsz]
        return np.ascontiguousarray(
            w.reshape(8, 128, nblk, nsz).transpose(2, 1, 0, 3))

    common = {
        "wqr": bf16(col_blocks(wq_e, 8, 128)),
        "wkr": bf16(col_blocks(wk_e, 8, 128)),
        "wvr": bf16(col_blocks(wv_e, 2, 512)),
        "wo": bf16(inputs["wo"]),
        "w1r": fp8(col_blocks(w1_e, 32, 128)),
        "w2r": fp8(np.asarray(inputs["w2"], dtype=np.float32)
           .reshape(16, 2, 128, D).transpose(0, 2, 1, 3)),
        "bq": f32(inputs["bq"] + b1l @ wq),
        "bk": f32(inputs["bk"] + b1l @ wk),
        "bv": f32(inputs["bv"] + b1l @ wv),
        "bo": f32(inputs["bo"]),
        "b1": f32(inputs["b1"] + b2l @ w1),
        "b2": f32(inputs["b2"]),
    }
    in_maps = []
    for c in range(NCORES):
        b = c // 4
        qoff = (c % 4) * QT
        m = dict(common)
        m["xq"] = np.ascontiguousarray(x[b, qoff:qoff + QT])
        m["xqt"] = np.ascontiguousarray(x[b, qoff:qoff + QT].T)
        in_maps.append(m)
    return in_maps


def kernel(x, ln1_g, ln1_b, wq, bq, wk, bk, wv, bv, wo, bo, w1, b1, w2, b2,
           ln2_g, ln2_b):
    inputs = {
        "x": x, "ln1_g": ln1_g, "ln1_b": ln1_b,
        "wq": wq, "bq": bq, "wk": wk, "bk": bk, "wv": wv, "bv": bv,
        "wo": wo, "bo": bo, "w1": w1, "b1": b1, "w2": w2, "b2": b2,
        "ln2_g": ln2_g, "ln2_b": ln2_b,
    }
    in_maps = make_in_maps(inputs)
    nc = _get_nc()
    res = run_bass_kernel_spmd(nc, in_maps, core_ids=list(range(NCORES)))

    y = np.empty((B, S, D), dtype=np.float32)
    for c in range(NCORES):
        b = c // 4
        qoff = (c % 4) * QT
        y[b, qoff:qoff + QT] = res.results[c]["y"]
    return y


# revision 25
# speedup vs baseline: 1.0741x; 1.0409x over previous
"""Transformer encoder layer (LN -> MHA -> residual -> LN -> MLP -> residual)
on 8 Trainium2 NeuronCores.

Sharding: token-parallel over the 4096 (batch*seq) tokens, 512 query-tokens
per core.  Each core computes K/V projections only for its OWN 512 tokens;
the full 2048-token K/V per batch is assembled with two AllGather
collectives (bf16, ~1 MB each) across the 4-core group sharing a batch.
Collectives run on TOPSP/SDMA and overlap with the Q projection.

On-chip layout: activations are kept feature-major ("transposed", [d, token])
so every matmul contracts along the partition dim.  Weights are pre-arranged
on the host so every weight DMA is one contiguous run per partition (DMA
descriptor count is the latency driver, not bytes).  Matmul operands are
bf16; accumulation stays fp32 in PSUM.  Softmax is computed unnormalized
(scores are bounded so plain exp is safe and algebraically identical); the
denominator comes from a ones-column interleaved into V before the gather,
and each head's attention accumulator stays resident in one PSUM bank
across all 16 k-tiles.  The score matmuls run one (wave, chunk) step ahead
of the attn@V matmuls so the exp's on the scalar engine pipeline behind
full-speed PE bursts.

LayerNorm gains/biases are folded into the following projections on the host
(exact algebra: (g*xhat+b) @ W = xhat @ (diag(g) W) + b @ W).
"""

import numpy as np
import ml_dtypes

import concourse.bass as bass
import concourse.mybir as mybir
from concourse import bacc
from concourse.tile import TileContext
from concourse.bass_utils import run_bass_kernel_spmd
from concourse.masks import make_identity

F32 = mybir.dt.float32
F32R = mybir.dt.float32r
BF16 = mybir.dt.bfloat16
FP8 = mybir.dt.float8e4
PM = mybir.MatmulPerfMode
AF = mybir.ActivationFunctionType
ALU = mybir.AluOpType

B, S, D = 2, 2048, 1024
H, HD = 16, 64
DFF = 4 * D
NCORES = 8
QT = 512            # query tokens per core
EPS = 1e-5
RG = [[0, 1, 2, 3], [4, 5, 6, 7]]  # replica groups (one per batch)


def _ln_stats(nc, lnp, eps, xt_a, xt_b):
    """bn stats over two [128, 512] token half-tiles -> (-mu*rstd, rstd)."""
    stats = lnp.tile([128, 2, 6], F32, tag="ln_st")
    nc.vector.bn_stats(stats[:, 0, :], xt_a)
    nc.vector.bn_stats(stats[:, 1, :], xt_b)
    mv = lnp.tile([128, 2], F32, tag="ln_mv")
    nc.vector.bn_aggr(mv, stats)
    sd = lnp.tile([128, 1], F32, tag="ln_sd")
    nc.scalar.activation(sd, mv[:, 1:2], AF.Sqrt, bias=eps[:, 0:1])
    rstd = lnp.tile([128, 1], F32, tag="ln_rs")
    nc.vector.reciprocal(rstd, sd)
    mr = lnp.tile([128, 2], F32R, tag="ln_mr")
    nc.vector.tensor_scalar(
        mr[:, 0:1], mv[:, 0:1], rstd, -1.0, ALU.mult, ALU.mult
    )
    nc.vector.tensor_copy(mr[:, 1:2], rstd)
    return mr


def _build():
    nc = bacc.Bacc(None, target_bir_lowering=False, num_devices=NCORES)

    XQ = nc.declare_dram_parameter("xq", [QT, D], F32, isOutput=False)
    XQT = nc.declare_dram_parameter("xqt", [D, QT], F32, isOutput=False)
    # host-prearranged weights: one contiguous run per partition per load
    WQR = nc.declare_dram_parameter("wqr", [8, 128, 8, 128], BF16, isOutput=False)
    WKR = nc.declare_dram_parameter("wkr", [8, 128, 8, 128], BF16, isOutput=False)
    WVR = nc.declare_dram_parameter("wvr", [2, 128, 8, 512], BF16, isOutput=False)
    WO = nc.declare_dram_parameter("wo", [D, D], BF16, isOutput=False)
    W1R = nc.declare_dram_parameter("w1r", [32, 128, 8, 128], FP8, isOutput=False)
    W2R = nc.declare_dram_parameter("w2r", [16, 128, 2, D], FP8, isOutput=False)
    BQ = nc.declare_dram_parameter("bq", [D], F32, isOutput=False)
    BK = nc.declare_dram_parameter("bk", [D], F32, isOutput=False)
    BV = nc.declare_dram_parameter("bv", [D], F32, isOutput=False)
    BO = nc.declare_dram_parameter("bo", [D], F32, isOutput=False)
    B1 = nc.declare_dram_parameter("b1", [DFF], F32, isOutput=False)
    B2 = nc.declare_dram_parameter("b2", [D], F32, isOutput=False)
    Y = nc.declare_dram_parameter("y", [QT, D], F32, isOutput=True)

    with TileContext(nc) as tc:
        with (
            tc.tile_pool(name="const", bufs=1) as cpool,
            tc.tile_pool(name="dram", bufs=1, space="DRAM") as dpool,
            tc.tile_pool(name="x2p", bufs=1) as x2p,
        ):
            ident32 = cpool.tile([128, 128], F32)
            make_identity(nc, ident32)
            ident16 = cpool.tile([128, 128], BF16)
            nc.vector.tensor_copy(ident16, ident32)
            eps = cpool.tile([128, 1], F32)
            nc.vector.memset(eps, EPS)
            ones64 = cpool.tile([1, 64], BF16)
            nc.vector.memset(ones64, 1.0)
            ones128f = cpool.tile([1, 128], F32)
            nc.vector.memset(ones128f, 1.0)
            ones128 = cpool.tile([1, 128], F32R)
            nc.vector.tensor_copy(ones128, ones128f)
            bqT = cpool.tile([128, 8], F32)
            nc.sync.dma_start(out=bqT, in_=BQ[:].rearrange("(t p) -> p t", p=128))
            bkT = cpool.tile([128, 8], F32)
            nc.sync.dma_start(out=bkT, in_=BK[:].rearrange("(t p) -> p t", p=128))
            b1T = cpool.tile([128, 32], F32)
            nc.sync.dma_start(out=b1T, in_=B1[:].rearrange("(t p) -> p t", p=128))
            bv_bc = cpool.tile([128, D], F32)
            nc.sync.dma_start(out=bv_bc, in_=BV[:].partition_broadcast(128))
            bo_bc = cpool.tile([128, D], F32)
            nc.sync.dma_start(out=bo_bc, in_=BO[:].partition_broadcast(128))
            b2_bc = cpool.tile([128, D], F32)
            nc.sync.dma_start(out=b2_bc, in_=B2[:].partition_broadcast(128))

            # DRAM bounce buffers for the K/V AllGathers (flat, partition-major)
            kT_loc = dpool.tile([128, 8 * QT], FP8)
            v_loc = dpool.tile([128, 4 * H * 65], FP8)
            kT_full = dpool.tile([4, 128, 8 * QT], FP8)
            v_full = dpool.tile([4, 128, 4 * H * 65], FP8)

            x2 = x2p.tile([128, 4, D], F32)     # post-attention residual
            attn128 = x2p.tile([128, 8, QT], BF16)  # normalized attn^T

            # ---- Phase 1: LN1 + K/V proj (own tokens) + gathers + Q proj ----
            with tc.tile_pool(name="qp", bufs=1) as qp:
              with (
                tc.tile_pool(name="kvloc", bufs=1) as kvp,
                tc.tile_pool(name="lnp", bufs=3) as lnp,
                tc.tile_pool(name="hTp", bufs=1) as hTp,
                tc.tile_pool(name="wsm", bufs=6) as wsm,
                tc.tile_pool(name="bcp", bufs=1) as bcp,
                tc.tile_pool(name="psT", bufs=2, space="PSUM") as psT,
                tc.tile_pool(name="psS1", bufs=1, space="PSUM") as psS1,
                tc.tile_pool(name="psK", bufs=3, space="PSUM") as psK,
              ):
                # -- LN1: stats in token space, apply in transposed space --
                hqT = hTp.tile([128, 8, QT], BF16)
                mr_row = lnp.tile([1, 512], F32R, tag="mr_row")
                rs_row = lnp.tile([1, 512], F32R, tag="rs_row")
                for st in range(4):
                    xt_a = lnp.tile([128, 512], F32, tag="ln_xa")
                    nc.sync.dma_start(
                        out=xt_a, in_=XQ[st * 128:(st + 1) * 128, 0:512])
                    xt_b = lnp.tile([128, 512], F32, tag="ln_xb")
                    nc.sync.dma_start(
                        out=xt_b, in_=XQ[st * 128:(st + 1) * 128, 512:1024])
                    mr = _ln_stats(nc, lnp, eps, xt_a, xt_b)
                    pst = psT.tile([128, 128], F32, tag="tp")
                    nc.tensor.transpose(pst[0:1, :], mr[:, 0:1].bitcast(F32),
                                        ident32)
                    nc.vector.tensor_copy(
                        mr_row[:, st * 128:(st + 1) * 128], pst[0:1, :])
                    pst2 = psT.tile([128, 128], F32, tag="tp")
                    nc.tensor.transpose(pst2[0:1, :], mr[:, 1:2].bitcast(F32),
                                        ident32)
                    nc.vector.tensor_copy(
                        rs_row[:, st * 128:(st + 1) * 128], pst2[0:1, :])
                # broadcast across partitions via rank-1 matmuls (f32r)
                mr_ps = psS1.tile([128, 512], F32, tag="psS")
                nc.tensor.matmul(mr_ps, ones128, mr_row, start=True, stop=True)
                mr_bc = bcp.tile([128, 512], F32, tag="mr")
                nc.vector.tensor_copy(mr_bc, mr_ps)
                rs_ps = psS1.tile([128, 512], F32, tag="psS")
                nc.tensor.matmul(rs_ps, ones128, rs_row, start=True, stop=True)
                rs_bc = bcp.tile([128, 512], F32, tag="rs")
                nc.vector.tensor_copy(rs_bc, rs_ps)
                for dt in range(8):
                    xtt = lnp.tile([128, 512], F32, tag="ln_xt")
                    nc.sync.dma_start(
                        out=xtt, in_=XQT[dt * 128:(dt + 1) * 128, :])
                    nc.vector.tensor_mul(xtt, xtt, rs_bc)
                    nc.vector.tensor_add(hqT[:, dt, :], xtt, mr_bc)

                # -- K proj -> feature-major [d, t] + bias, then gather --
                kloc_sb = kvp.tile([128, 8, QT], FP8)
                for ht in range(8):
                    wcol = wsm.tile([128, 8, 128], BF16, tag="w")
                    nc.sync.dma_start(out=wcol, in_=WKR[ht])
                    psk = psK.tile([128, 512], F32, tag="psK")
                    for dt in range(8):
                        nc.tensor.matmul(
                            psk, wcol[:, dt, :], hqT[:, dt, :],
                            start=(dt == 0), stop=(dt == 7),
                        )
                    nc.vector.tensor_scalar_add(
                        kloc_sb[:, ht, :], psk, bkT[:, ht:ht + 1]
                    )
                nc.sync.dma_start(
                    out=kT_loc[:, :],
                    in_=kloc_sb[:].rearrange("p t n -> p (t n)"),
                )
                nc.gpsimd.collective_compute(
                    "AllGather", ALU.bypass, replica_groups=RG,
                    ins=[kT_loc.opt()], outs=[kT_full.opt()],
                )

                # -- V proj -> token-major [t, (h 65)] + bias + ones col --
                vloc_sb = kvp.tile([128, 4, H, 65], FP8)
                nc.vector.memset(vloc_sb[:, :, :, 64:65], 1.0)
                for hc in range(2):
                    wv_sb = wsm.tile([128, 8, 512], BF16, tag="wv",
                                     name=f"wv{hc}")
                    for dq in range(4):
                        nc.sync.dma_start(
                            out=wv_sb[:, dq * 2:(dq + 1) * 2, :],
                            in_=WVR[hc, :, dq * 2:(dq + 1) * 2, :],
                        )
                    for st in range(4):
                        psv = psK.tile([128, 512], F32, tag="psK")
                        for dt in range(8):
                            nc.tensor.matmul(
                                psv,
                                hqT[:, dt, st * 128:(st + 1) * 128],
                                wv_sb[:, dt, :],
                                start=(dt == 0), stop=(dt == 7),
                            )
                        nc.vector.tensor_add(
                            vloc_sb[:, st, hc * 8:(hc + 1) * 8, 0:64],
                            psv.rearrange("p (h d) -> p h d", h=8),
                            bv_bc[:, hc * 512:(hc + 1) * 512].rearrange(
                                "p (h d) -> p h d", h=8),
                        )
                nc.scalar.dma_start(
                    out=v_loc[:, :],
                    in_=vloc_sb[:].rearrange("p s h d -> p (s h d)"),
                )
                nc.gpsimd.collective_compute(
                    "AllGather", ALU.bypass, replica_groups=RG,
                    ins=[v_loc.opt()], outs=[v_full.opt()],
                )

                # -- Q proj (overlaps with the gathers) --
                Q_sb = qp.tile([128, 8, QT], FP8)
                for ht in range(8):
                    wcol = wsm.tile([128, 8, 128], BF16, tag="w")
                    nc.sync.dma_start(out=wcol, in_=WQR[ht])
                    psq = psK.tile([128, 512], F32, tag="psK")
                    for dt in range(8):
                        nc.tensor.matmul(
                            psq, wcol[:, dt, :], hqT[:, dt, :],
                            start=(dt == 0), stop=(dt == 7),
                        )
                    nc.vector.tensor_scalar_add(
                        Q_sb[:, ht, :], psq, bqT[:, ht:ht + 1]
                    )

              # ---- Phase 2+3: load gathered K/V, pipelined attention ----
              with (
                  tc.tile_pool(name="kvall", bufs=1) as kva,
                  tc.tile_pool(name="pp", bufs=20) as ppl,
                  tc.tile_pool(name="accw", bufs=2) as accwp,
                  tc.tile_pool(name="dnp", bufs=2) as dnp,
                  tc.tile_pool(name="psS", bufs=2, space="PSUM") as psS,
                  tc.tile_pool(name="psA", bufs=4, space="PSUM") as psA,
              ):
                  KT_all = kva.tile([128, 4, 8, QT], FP8)
                  V_all = kva.tile([128, 4, 4, H, 65], FP8)
                  for g in range(4):
                      for half in range(2):
                          nc.sync.dma_start(
                              out=KT_all[:, g, half * 4:(half + 1) * 4, :],
                              in_=kT_full[g][:, half * 2048:(half + 1) * 2048]
                              .rearrange("p (t n) -> p t n", t=4),
                          )
                  for g in range(4):
                      for q in range(4):
                          nc.scalar.dma_start(
                              out=V_all[:, g, q, :, :],
                              in_=v_full[g][:, q * 1040:(q + 1) * 1040]
                              .rearrange("p (h d) -> p h d", h=H),
                          )

                  # per (wave, chunk) step: 16 score matmuls + 8 paired exps
                  # + 16 attn@V matmuls; scores run one step ahead; each
                  # wave's attention accumulators are normalized in-phase
                  steps = [(w, g) for w in range(4) for g in range(4)]
                  psa = {}
                  p_tiles = {}

                  def emit_scores(w, g):
                      for h in range(w * 4, w * 4 + 4):
                          kj, ko = h // 2, (h % 2) * 64
                          for pair in range(2):
                              pss2 = psS.tile([128, 2, 512], F32, tag="psS",
                                              name=f"pss{h}_{g}_{pair}")
                              for j in range(2):
                                  ktl = pair * 2 + j
                                  nc.tensor.matmul(
                                      pss2[:, j, :],
                                      KT_all[ko:ko + 64, g, kj,
                                             ktl * 128:(ktl + 1) * 128],
                                      Q_sb[ko:ko + 64, kj, :],
                                      start=True, stop=True,
                                      perf_mode=PM.DoublePixel,
                                  )
                              P2 = ppl.tile([128, 2, 512], FP8, tag="P",
                                            name=f"P{h}_{g}_{pair}")
                              nc.scalar.activation(P2, pss2, AF.Exp,
                                                   scale=0.125)
                              p_tiles[(h, g, pair)] = P2

                  def emit_attnv(w, g):
                      for h in range(w * 4, w * 4 + 4):
                          if g == 0:
                              psa[h] = psA.tile([65, 512], F32, tag="psA",
                                                name=f"psa{h}")
                          for pair in range(2):
                              P2 = p_tiles.pop((h, g, pair))
                              for j in range(2):
                                  ktl = pair * 2 + j
                                  nc.tensor.matmul(
                                      psa[h],
                                      V_all[:, g, ktl, h, :],
                                      P2[:, j, :],
                                      start=(g == 0 and ktl == 0),
                                      stop=(g == 3 and ktl == 3),
                                      perf_mode=PM.DoublePixel,
                                  )

                  def emit_normalize(w):
                      heads = list(range(w * 4, w * 4 + 4))
                      aw = accwp.tile([65, 4, 512], BF16, tag="aw",
                                      name=f"aw{w}")
                      for i, h in enumerate(heads):
                          nc.vector.tensor_copy(aw[:, i, :], psa.pop(h))
                      dn = dnp.tile([4, 512], BF16, tag="dn", name=f"dn{w}")
                      for i in range(4):
                          nc.sync.dma_start(out=dn[i:i + 1, :],
                                            in_=aw[64:65, i, :])
                      rw = dnp.tile([4, 512], F32, tag="rw", name=f"rw{w}")
                      nc.vector.reciprocal(rw, dn)
                      rwb = dnp.tile([4, 512], BF16, tag="rwb", name=f"rwb{w}")
                      nc.vector.tensor_copy(rwb, rw)
                      rrow = dnp.tile([1, 4, 512], BF16, tag="rrow",
                                      name=f"rrow{w}")
                      for i in range(4):
                          nc.sync.dma_start(out=rrow[0:1, i, :],
                                            in_=rwb[i:i + 1, :])
                      for i, h in enumerate(heads):
                          rbt = psS.tile([128, 2, 512], F32, tag="psS",
                                         name=f"rb{w}_{i}")
                          nc.tensor.matmul(rbt[0:64, 0, :], ones64,
                                           rrow[0:1, i, :],
                                           start=True, stop=True)
                          kj, ko = h // 2, (h % 2) * 64
                          nc.vector.tensor_mul(
                              attn128[ko:ko + 64, kj, :], aw[0:64, i, :],
                              rbt[0:64, 0, :]
                          )

                  emit_scores(*steps[0])
                  for i in range(len(steps)):
                      if i + 1 < len(steps):
                          emit_scores(*steps[i + 1])
                      emit_attnv(*steps[i])
                      if steps[i][1] == 3:
                          emit_normalize(steps[i][0])

            # ---- Phase 4: O proj + residual ----
            with (
                tc.tile_pool(name="xqp", bufs=1) as xqp,
                tc.tile_pool(name="dwo", bufs=8) as dwo,
                tc.tile_pool(name="dtmp", bufs=4) as dtmp,
                tc.tile_pool(name="psO", bufs=4, space="PSUM") as psO,
            ):
                xq_sb = xqp.tile([128, 4, D], F32)
                for st in range(4):
                    nc.sync.dma_start(
                        out=xq_sb[:, st, :],
                        in_=XQ[st * 128:(st + 1) * 128, :])
                for c in range(2):
                    po = [psO.tile([128, 512], F32, tag="psO",
                                   name=f"po{c}_{i}") for i in range(4)]
                    for j in range(8):
                        wot = dwo.tile([128, 512], BF16, tag="wo")
                        nc.sync.dma_start(
                            out=wot,
                            in_=WO[j * 128:(j + 1) * 128,
                                   c * 512:(c + 1) * 512],
                        )
                        for qt in range(4):
                            nc.tensor.matmul(
                                po[qt], attn128[:, j, qt * 128:(qt + 1) * 128],
                                wot, start=(j == 0), stop=(j == 7),
                            )
                    for qt in range(4):
                        t1 = dtmp.tile([128, 512], F32, tag="t1")
                        nc.vector.tensor_add(
                            t1, po[qt], bo_bc[:, c * 512:(c + 1) * 512]
                        )
                        nc.vector.tensor_add(
                            x2[:, qt, c * 512:(c + 1) * 512],
                            t1,
                            xq_sb[:, qt, c * 512:(c + 1) * 512],
                        )

            # ---- Phase 5: LN2 + MLP + residual ----
            with (
                tc.tile_pool(name="lnp2", bufs=3) as lnp2,
                tc.tile_pool(name="h2p", bufs=1) as h2p,
                tc.tile_pool(name="gp", bufs=1) as gp,
                tc.tile_pool(name="wfp", bufs=8) as wfp,
                tc.tile_pool(name="w2p", bufs=8) as w2p,
                tc.tile_pool(name="yp", bufs=2) as yp,
            ):
                h2T = h2p.tile([128, 8, QT], FP8)
                G = gp.tile([128, 32, QT], FP8)
                with (
                    tc.tile_pool(name="psT2", bufs=2, space="PSUM") as psT2,
                    tc.tile_pool(name="psF", bufs=4, space="PSUM") as psF,
                ):
                    # LN2 from SBUF-resident x2, transposed output
                    for st in range(4):
                        mr = _ln_stats(nc, lnp2, eps,
                                       x2[:, st, 0:512], x2[:, st, 512:1024])
                        h2 = lnp2.tile([128, D], BF16, tag="ln_h")
                        # h2 = x2*rstd + (-mu*rstd)
                        nc.vector.tensor_scalar(
                            h2, x2[:, st, :], mr[:, 1:2].bitcast(F32),
                            mr[:, 0:1].bitcast(F32), ALU.mult, ALU.add
                        )
                        for dt in range(8):
                            pst = psT2.tile([128, 128], BF16, tag="tp2")
                            nc.tensor.transpose(
                                pst, h2[:, dt * 128:(dt + 1) * 128], ident16
                            )
                            nc.vector.tensor_copy(
                                h2T[:, dt, st * 128:(st + 1) * 128], pst
                            )

                    # MLP1: gelu(h2 @ w1 + b1), transposed output [dff, q]
                    for ft in range(32):
                        w1c = wfp.tile([128, 8, 128], FP8, tag="w1")
                        nc.sync.dma_start(out=w1c, in_=W1R[ft])
                        psf = psF.tile([128, 512], F32, tag="psF")
                        for j in range(4):
                            nc.tensor.matmul(
                                psf, w1c[:, 2 * j:2 * j + 2, :],
                                h2T[:, 2 * j:2 * j + 2, :],
                                start=(j == 0), stop=(j == 3),
                                perf_mode=PM.DoubleRow,
                            )
                        nc.scalar.activation(
                            G[:, ft, :], psf, AF.Gelu, bias=b1T[:, ft:ft + 1]
                        )

                # MLP2: y = G^T @ w2 + b2 + x2
                with tc.tile_pool(name="psY", bufs=4, space="PSUM") as psY:
                    for c in range(2):
                        py = [psY.tile([128, 512], F32, tag="psY",
                                       name=f"py{c}_{i}") for i in range(4)]
                        for jp in range(16):
                            w2t = w2p.tile([128, 2, 512], FP8, tag="w2")
                            nc.sync.dma_start(
                                out=w2t,
                                in_=W2R[jp, :, :, c * 512:(c + 1) * 512],
                            )
                            for qt in range(4):
                                nc.tensor.matmul(
                                    py[qt],
                                    G[:, 2 * jp:2 * jp + 2,
                                      qt * 128:(qt + 1) * 128],
                                    w2t, start=(jp == 0), stop=(jp == 15),
                                    perf_mode=PM.DoubleRow,
                                )
                        for qt in range(4):
                            t1 = yp.tile([128, 512], F32, tag="yt1")
                            nc.vector.tensor_add(
                                t1, py[qt], b2_bc[:, c * 512:(c + 1) * 512]
                            )
                            yt = yp.tile([128, 512], F32, tag="yt2")
                            nc.vector.tensor_add(
                                yt, t1, x2[:, qt, c * 512:(c + 1) * 512]
                            )
                            nc.sync.dma_start(
                                out=Y[qt * 128:(qt + 1) * 128,
                                      c * 512:(c + 1) * 512],
                                in_=yt,
                            )

    nc.compile()
    return nc


_NC = None


def _get_nc():
    global _NC
    if _NC is None:
        _NC = _build()
    return _NC


def make_in_maps(inputs):
    f32 = lambda a: np.ascontiguousarray(np.asarray(a, dtype=np.float32))
    bf16 = lambda a: np.ascontiguousarray(
        np.asarray(a, dtype=np.float32).astype(ml_dtypes.bfloat16)
    )
    fp8 = lambda a: np.ascontiguousarray(
        np.asarray(a, dtype=np.float32).astype(ml_dtypes.float8_e4m3fn)
    )
    x = f32(inputs["x"])
    g1, b1l = f32(inputs["ln1_g"]), f32(inputs["ln1_b"])
    g2, b2l = f32(inputs["ln2_g"]), f32(inputs["ln2_b"])
    wq, wk, wv = f32(inputs["wq"]), f32(inputs["wk"]), f32(inputs["wv"])
    w1 = f32(inputs["w1"])

    # Fold LayerNorm affine params into the following projections (exact).
    wq_e = g1[:, None] * wq
    wk_e = g1[:, None] * wk
    wv_e = g1[:, None] * wv
    w1_e = g2[:, None] * w1

    # host pre-layouts: [out-block, partition, dt, n]
    def col_blocks(w, nblk, nsz):
        # w [1024, nblk*nsz] -> [nblk, 128, 8, nsz]
        return np.ascontiguousarray(
            w.reshape(8, 128, nblk, nsz).transpose(2, 1, 0, 3))

    common = {
        "wqr": bf16(col_blocks(wq_e, 8, 128)),
        "wkr": bf16(col_blocks(wk_e, 8, 128)),
        "wvr": bf16(col_blocks(wv_e, 2, 512)),
        "wo": bf16(inputs["wo"]),
        "w1r": fp8(col_blocks(w1_e, 32, 128)),
        "w2r": fp8(np.asarray(inputs["w2"], dtype=np.float32)
           .reshape(16, 2, 128, D).transpose(0, 2, 1, 3)),
        "bq": f32(inputs["bq"] + b1l @ wq),
        "bk": f32(inputs["bk"] + b1l @ wk),
        "bv": f32(inputs["bv"] + b1l @ wv),
        "bo": f32(inputs["bo"]),
        "b1": f32(inputs["b1"] + b2l @ w1),
        "b2": f32(inputs["b2"]),
    }
    in_maps = []
    for c in range(NCORES):
        b = c // 4
        qoff = (c % 4) * QT
        m = dict(common)
        m["xq"] = np.ascontiguousarray(x[b, qoff:qoff + QT])
        m["xqt"] = np.ascontiguousarray(x[b, qoff:qoff + QT].T)
        in_maps.append(m)
    return in_maps


def kernel(x, ln1_g, ln1_b, wq, bq, wk, bk, wv, bv, wo, bo, w1, b1, w2, b2,
           ln2_g, ln2_b):
    inputs = {
        "x": x, "ln1_g": ln1_g, "ln1_b": ln1_b,
        "wq": wq, "bq": bq, "wk": wk, "bk": bk, "wv": wv, "bv": bv,
        "wo": wo, "bo": bo, "w1": w1, "b1": b1, "w2": w2, "b2": b2,
        "ln2_g": ln2_g, "ln2_b": ln2_b,
    }
    in_maps = make_in_maps(inputs)
    nc = _get_nc()
    res = run_bass_kernel_spmd(nc, in_maps, core_ids=list(range(NCORES)))

    y = np.empty((B, S, D), dtype=np.float32)
    for c in range(NCORES):
        b = c // 4
        qoff = (c % 4) * QT
        y[b, qoff:qoff + QT] = res.results[c]["y"]
    return y
